# revision 12
# baseline (speedup 1.0000x reference)
"""Trainium2 Bass kernel for the two-stream LSTM encoder.

Strategy (8 NeuronCores, data-parallel over batch):
  - Each core gets B/8 = 16 batch elements; all weights replicated.
  - Everything on-device runs in "T-layout": channels on SBUF partitions,
    (batch, time) on the free dimension. fp16 matmul operands, fp32 psum
    accumulation and elementwise/state math (validated: rel err ~5e-4).
  - Phase A: z = feat @ w_emb.T streamed over row-chunks (PE transposes the
    activations into T-layout), per-channel sum/sumsq accumulated, raw z
    spilled to DRAM as fp16. BatchNorm statistics are all-reduced across the
    8 cores (training-mode BN over the full 32768-row batch).
  - Phase B: time loop in chunks of 16 steps. Per chunk: z chunk -> BN+ReLU
    (one ACT op per d-chunk, per-partition scale/bias) -> input projections
    u = e @ w_ih.T batched over the chunk (bias folded in via a K=1 matmul
    against a ones row). Per step: only the two h @ w_hh.T cell matmuls
    (64 weight-stationary 128x128 MMs each) + ACT sigmoid/tanh + DVE state
    update. Cross-stream output gates, the final fusion matmul and the
    transpose back to natural layout are batched per chunk (they lag the
    recurrence and hide in the PE array gaps).
Gate order is host-permuted from torch's (i,f,g,o) to (i,f,o,g) so one
sigmoid ACT covers a contiguous [128,192] region and one tanh covers [128,64].
"""

import os
import numpy as np

os.environ.setdefault("MYCRO_LOCAL_CACHE", "1")

NCORES = 8
B, T, D, F0, F1 = 128, 256, 512, 2048, 1024
BS = B // NCORES          # 16 batch rows per core
ROWS = BS * T             # 4096
TC = 16                   # time steps per chunk
NCHUNK = T // TC
EPS = 1e-5
G4 = 4 * D                # 2048 gate dim
NDC = D // 128            # 4 d-chunks
NGC = G4 // 128           # 16 gate chunks
NRC = ROWS // 128         # 32 row chunks

_BUILT = None


def _build(t_steps=T, use_collective=True, stage="full"):
    import concourse.bass as bass
    import concourse.bacc as bacc
    import concourse.mybir as mybir
    import concourse.tile as tile
    from concourse.masks import make_identity
    from contextlib import ExitStack

    f16 = mybir.dt.float16
    f32 = mybir.dt.float32
    f8 = mybir.dt.float8e4
    AF = mybir.ActivationFunctionType
    ALU = mybir.AluOpType

    nchunk = t_steps // TC
    rows = BS * t_steps
    nrc = rows // 128

    nc = bacc.Bacc(None, num_devices=NCORES)

    # ---------------- DRAM parameters ----------------
    feat0 = nc.declare_dram_parameter("feat0", [rows, F0], f32, isOutput=False)
    feat1 = nc.declare_dram_parameter("feat1", [rows, F1], f32, isOutput=False)
    maskp = nc.declare_dram_parameter("feat_mask", [BS, t_steps], f32, isOutput=False)
    wemb0T = nc.declare_dram_parameter("w_emb0T", [F0, D], f16, isOutput=False)
    wemb1T = nc.declare_dram_parameter("w_emb1T", [F1, D], f16, isOutput=False)
    wih0T = nc.declare_dram_parameter("w_ih0T", [D, G4], f16, isOutput=False)
    whh0T = nc.declare_dram_parameter("w_hh0T", [D, G4], f8, isOutput=False)
    wih1T = nc.declare_dram_parameter("w_ih1T", [D, G4], f16, isOutput=False)
    whh1T = nc.declare_dram_parameter("w_hh1T", [D, G4], f8, isOutput=False)
    wg0T = nc.declare_dram_parameter("wg0T", [D, D], f16, isOutput=False)
    wg1T = nc.declare_dram_parameter("wg1T", [D, D], f16, isOutput=False)
    wf1T = nc.declare_dram_parameter("wf1T", [D, D], f16, isOutput=False)
    wf2T = nc.declare_dram_parameter("wf2T", [D, D], f16, isOutput=False)
    bc0p = nc.declare_dram_parameter("bc0", [1, G4], f16, isOutput=False)
    bc1p = nc.declare_dram_parameter("bc1", [1, G4], f16, isOutput=False)
    bg0p = nc.declare_dram_parameter("bg0", [1, D], f16, isOutput=False)
    bg1p = nc.declare_dram_parameter("bg1", [1, D], f16, isOutput=False)
    bfp = nc.declare_dram_parameter("bf", [1, D], f16, isOutput=False)
    gamma0p = nc.declare_dram_parameter("gamma0", [D], f32, isOutput=False)
    beta0p = nc.declare_dram_parameter("beta0", [D], f32, isOutput=False)
    gamma1p = nc.declare_dram_parameter("gamma1", [D], f32, isOutput=False)
    beta1p = nc.declare_dram_parameter("beta1", [D], f32, isOutput=False)
    out_d = nc.declare_dram_parameter("out", [BS, t_steps, D], f32, isOutput=True)

    # DRAM scratch: z in T-layout [dc, d_in_chunk, rows], fp16
    z0d = nc.dram_tensor("z0d", [NDC, 128, rows], f16)
    z1d = nc.dram_tensor("z1d", [NDC, 128, rows], f16)
    st_in = nc.dram_tensor("st_in", [128, 16], f32)
    st_out = nc.dram_tensor("st_out", [128, 16], f32)

    inv_n = 1.0 / float(B * t_steps)
    if not use_collective:
        inv_n = 1.0 / float(BS * t_steps)

    with tile.TileContext(nc) as tc, ExitStack() as stk:
        wp = stk.enter_context(tc.tile_pool(name="wp", bufs=1))

        # ---------------- resident small tiles ----------------
        ident = wp.tile([128, 128], f16)
        make_identity(nc, ident)
        ones16 = wp.tile([1, 512], f16)
        nc.vector.memset(ones16, 1.0)
        ones32 = wp.tile([1, 128], f32)
        nc.vector.memset(ones32, 1.0)
        eps_t = wp.tile([128, 1], f32)
        nc.vector.memset(eps_t, 0.0)

        # phase-B weight loads are deferred until after phase A's feat DMAs
        # are queued, so they don't steal HBM bandwidth from the critical
        # first feat tiles.
        def load_T(dram, nch, free):
            t = wp.tile([128, nch, free], f16, name=f"w_{dram.name}")
            nc.sync.dma_start(
                out=t, in_=dram.rearrange("(c p) g -> p c g", p=128))
            return t

        _STAGES = ("w", "feat", "tr", "mm", "a1", "a2", "a3", "a", "bn", "u", "steps", "full")
        def _ge(x):
            return _STAGES.index(stage) >= _STAGES.index(x)

        # per-rowchunk stat slots: col = dc*nrc + rc
        stat = {}
        for s in range(2):
            stat[(s, "s")] = wp.tile([128, NDC * nrc], f32, name=f"stat_s{s}")
            stat[(s, "q")] = wp.tile([128, NDC * nrc], f32, name=f"stat_q{s}")

        # ---------------- Phase A: embeddings + stats ----------------
        with tc.tile_pool(name="pa", bufs=2) as pa, \
             tc.tile_pool(name="paw", bufs=1) as paw, \
             tc.tile_pool(name="psA", bufs=1, space="PSUM") as psA:
            we0 = paw.tile([128, F0 // 128, D], f16)
            nc.sync.dma_start(
                out=we0, in_=wemb0T.rearrange("(c p) g -> p c g", p=128))
            we1 = paw.tile([128, F1 // 128, D], f16)
            nc.sync.dma_start(
                out=we1, in_=wemb1T.rearrange("(c p) g -> p c g", p=128))

            for s, (featp, nf, wemb, zdram) in enumerate(
                    ((feat0, F0, we0, z0d), (feat1, F1, we1, z1d))):
                nfc = nf // 128
                for rc in range(nrc if _ge("feat") else 0):
                    ftile = pa.tile([128, nf], f32, tag=f"ft{s}")
                    nc.gpsimd.dma_start(
                        out=ftile, in_=featp[rc * 128:(rc + 1) * 128, :])
                    f16t = pa.tile([128, nf], f16, tag=f"f16_{s}")
                    nc.scalar.copy(f16t, ftile)
                    if not _ge("tr"):
                        continue
                    fT = pa.tile([128, nfc, 128], f16, tag=f"fT{s}")
                    for fc in range(nfc):
                        tp = psA.tile([128, 128], f16, tag="tp", bufs=4)
                        nc.tensor.transpose(
                            tp, f16t[:, fc * 128:(fc + 1) * 128], ident)
                        nc.vector.tensor_copy(fT[:, fc], tp)
                    if not _ge("mm"):
                        continue
                    za = psA.tile([128, D], f32, tag="za", bufs=2)
                    for dc in range(NDC):
                        for fc in range(nfc):
                            nc.tensor.matmul(
                                za[:, dc * 128:(dc + 1) * 128],
                                lhsT=wemb[:, fc, dc * 128:(dc + 1) * 128],
                                rhs=fT[:, fc],
                                start=(fc == 0), stop=(fc == nfc - 1))
                    if not _ge("a1"):
                        continue
                    zst = pa.tile([128, NDC, 128], f16, tag=f"zst{s}")
                    sq = pa.tile([128, 128], f32, tag="sq")
                    for dc in range(NDC):
                        nc.scalar.activation(
                            zst[:, dc], za[:, dc * 128:(dc + 1) * 128],
                            AF.Identity,
                            accum_out=stat[(s, "s")][:, dc * nrc + rc:dc * nrc + rc + 1])
                        if _ge("a2"):
                            nc.vector.tensor_tensor(
                                sq, za[:, dc * 128:(dc + 1) * 128],
                                zst[:, dc], op=ALU.mult)
                            nc.vector.reduce_sum(
                                stat[(s, "q")][:, dc * nrc + rc:dc * nrc + rc + 1],
                                sq, axis=mybir.AxisListType.X)
                    if _ge("a3"):
                        nc.gpsimd.dma_start(
                            out=zdram[:, :, rc * 128:(rc + 1) * 128].rearrange(
                                "c p r -> p c r"),
                            in_=zst)

        # deferred phase-B weight DMAs (sync queue, overlap with phase A tail)
        wih0 = load_T(wih0T, NDC, G4)
        whh0 = load_T(whh0T, NDC, G4)
        wih1 = load_T(wih1T, NDC, G4)
        whh1 = load_T(whh1T, NDC, G4)
        wg0 = load_T(wg0T, NDC, D)
        wg1 = load_T(wg1T, NDC, D)
        wf1 = load_T(wf1T, NDC, D)
        wf2 = load_T(wf2T, NDC, D)
        bc0 = wp.tile([1, G4], f16)
        nc.sync.dma_start(out=bc0, in_=bc0p[:, :])
        bc1 = wp.tile([1, G4], f16)
        nc.sync.dma_start(out=bc1, in_=bc1p[:, :])
        bg0 = wp.tile([1, D], f16)
        nc.sync.dma_start(out=bg0, in_=bg0p[:, :])
        bg1 = wp.tile([1, D], f16)
        nc.sync.dma_start(out=bg1, in_=bg1p[:, :])
        bf = wp.tile([1, D], f16)
        nc.sync.dma_start(out=bf, in_=bfp[:, :])

        gam0 = wp.tile([128, NDC], f32)
        nc.sync.dma_start(out=gam0, in_=gamma0p.rearrange("(c p) -> p c", p=128))
        bet0 = wp.tile([128, NDC], f32)
        nc.sync.dma_start(out=bet0, in_=beta0p.rearrange("(c p) -> p c", p=128))
        gam1 = wp.tile([128, NDC], f32)
        nc.sync.dma_start(out=gam1, in_=gamma1p.rearrange("(c p) -> p c", p=128))
        bet1 = wp.tile([128, NDC], f32)
        nc.sync.dma_start(out=bet1, in_=beta1p.rearrange("(c p) -> p c", p=128))

        # mask (fp32, consumed by K=1 broadcast matmuls per chunk)
        mflat = wp.tile([1, BS, t_steps], f32)
        nc.sync.dma_start(out=mflat[0:1], in_=maskp[:, :])

        # ---------------- BN stats allreduce + scale/shift ----------------
        ared = wp.tile([128, 16], f32)
        for s in range(2 if _ge("a") else 0):
            nc.vector.reduce_sum(
                ared[:, s * 8:s * 8 + 4],
                stat[(s, "s")].rearrange("p (c r) -> p c r", c=NDC),
                axis=mybir.AxisListType.X)
            nc.vector.reduce_sum(
                ared[:, s * 8 + 4:s * 8 + 8],
                stat[(s, "q")].rearrange("p (c r) -> p c r", c=NDC),
                axis=mybir.AxisListType.X)
        if _ge("a"):
            nc.gpsimd.dma_start(out=st_in[:, :], in_=ared)
        if use_collective and _ge("bn"):
            nc.gpsimd.collective_compute(
                "AllReduce", ALU.add,
                replica_groups=[list(range(NCORES))],
                ins=[st_in[:, :]], outs=[st_out[:, :]])
        else:
            nc.gpsimd.dma_start(out=st_out[:, :], in_=st_in[:, :])
        ag = wp.tile([128, 16], f32)
        nc.gpsimd.dma_start(out=ag, in_=st_out[:, :])
        bn_on = _ge("bn")

        # a = gamma / sqrt(var+eps), c = beta - mu * a   (per stream)
        bn_a, bn_c = [], []
        for s, (gam, bet) in enumerate(
                ((gam0, bet0), (gam1, bet1)) if bn_on else ()):
            mu = wp.tile([128, NDC], f32, name=f"mu{s}")
            nc.vector.tensor_scalar_mul(mu, ag[:, s * 8:s * 8 + 4], inv_n)
            var = wp.tile([128, NDC], f32, name=f"var{s}")
            nc.vector.tensor_scalar_mul(var, ag[:, s * 8 + 4:s * 8 + 8], inv_n)
            musq = wp.tile([128, NDC], f32, name=f"musq{s}")
            nc.vector.tensor_mul(musq, mu, mu)
            nc.vector.tensor_sub(var, var, musq)
            nc.vector.tensor_scalar_add(var, var, EPS)
            sig = wp.tile([128, NDC], f32, name=f"sig{s}")
            nc.scalar.activation(sig, var, AF.Sqrt, bias=eps_t[:, 0:1])
            isig = wp.tile([128, NDC], f32, name=f"isig{s}")
            nc.vector.reciprocal(isig, sig)
            a_t = wp.tile([128, NDC], f32, name=f"bna{s}")
            nc.vector.tensor_mul(a_t, gam, isig)
            c_t = wp.tile([128, NDC], f32, name=f"bnc{s}")
            nc.vector.tensor_mul(c_t, mu, a_t)
            nc.vector.tensor_sub(c_t, bet, c_t)
            bn_a.append(a_t)
            bn_c.append(c_t)

        # ---------------- Phase B: recurrence ----------------
        pb = stk.enter_context(tc.tile_pool(name="pb", bufs=2))
        ps = stk.enter_context(tc.tile_pool(name="ps", bufs=2, space="PSUM"))

        # persistent state
        h_zero = wp.tile([128, NDC, BS], f16)
        nc.vector.memset(h_zero, 0.0)
        c_state = []
        for s in range(2):
            cs = wp.tile([128, NDC, BS], f32, name=f"cstate{s}")
            nc.vector.memset(cs, 0.0)
            c_state.append(cs)
        h_prev = [h_zero, h_zero]

        whh = (whh0, whh1)
        wih = (wih0, wih1)
        bc = (bc0, bc1)

        for c in range(nchunk if _ge("u") else 0):
            t0 = c * TC
            # -- load z chunks, BN+ReLU -> e (fp16) --
            ec = []
            for s, zdram in enumerate((z0d, z1d)):
                zc = pb.tile([128, NDC, BS, TC], f16, tag=f"zc{s}")
                for dc in range(NDC):
                    nc.sync.dma_start(
                        out=zc[:, dc],
                        in_=zdram.rearrange("c p (b t) -> p c b t", b=BS)[
                            :, dc, :, t0:t0 + TC])
                e = pb.tile([128, NDC, BS, TC], f16, tag=f"ec{s}")
                for dc in range(NDC):
                    nc.scalar.activation(
                        e[:, dc], zc[:, dc], AF.Relu,
                        bias=bn_c[s][:, dc:dc + 1],
                        scale=bn_a[s][:, dc:dc + 1])
                ec.append(e)

            # -- input projections u = e @ w_ih.T + bc (batched, K=1 bias) --
            u = []
            for s in range(2):
                ut = pb.tile([128, NGC, BS, TC], f16, tag=f"u{s}")
                for g in range(NGC):
                    up = ps.tile([128, BS, TC], f32, tag="u")
                    for dc in range(NDC):
                        nc.tensor.matmul(
                            up,
                            lhsT=wih[s][:, dc, g * 128:(g + 1) * 128],
                            rhs=ec[s][:, dc],
                            start=(dc == 0), stop=False)
                    nc.tensor.matmul(
                        up, lhsT=bc[s][0:1, g * 128:(g + 1) * 128],
                        rhs=ones16[0:1, 0:BS * TC],
                        start=False, stop=True)
                    nc.vector.tensor_copy(ut[:, g], up)
                u.append(ut)

            # -- mask broadcast for this chunk (fp32 K=1 matmuls, one per dc) --
            mp = ps.tile([128, NDC, BS, TC], f32, tag="lag")
            for dc in range(NDC):
                nc.tensor.matmul(
                    mp[:, dc],
                    lhsT=ones32[0:1, :],
                    rhs=mflat[0:1, :, t0:t0 + TC],
                    start=True, stop=True)
            msk = pb.tile([128, NDC, BS, TC], f16, tag="msk")
            nc.vector.tensor_copy(msk, mp)

            # -- recurrence steps --
            if not _ge("steps"):
                continue
            hh_t = [pb.tile([128, TC, NDC, BS], f16, tag="hh0", bufs=3,
                            name="hh0"),
                    pb.tile([128, TC, NDC, BS], f16, tag="hh1", bufs=3,
                            name="hh1")]
            for tl in range(TC):
                for s in range(2):
                    gp = ps.tile([128, NGC, BS], f32, tag="g")
                    for g in range(NGC):
                        for dc in range(NDC):
                            nc.tensor.matmul(
                                gp[:, g],
                                lhsT=whh[s][:, dc, g * 128:(g + 1) * 128],
                                rhs=h_prev[s][:, dc],
                                start=(dc == 0), stop=(dc == NDC - 1))
                    gsb = pb.tile([128, NGC, BS], f32, tag="gsb")
                    nc.vector.tensor_tensor(
                        gsb, gp, u[s][:, :, :, tl], op=ALU.add)
                    sg = pb.tile([128, 3 * NDC, BS], f32, tag="sg")
                    nc.scalar.activation(sg, gsb[:, 0:3 * NDC], AF.Sigmoid)
                    tg = pb.tile([128, NDC, BS], f32, tag="tg")
                    nc.scalar.activation(tg, gsb[:, 3 * NDC:4 * NDC], AF.Tanh)
                    t1 = pb.tile([128, NDC, BS], f32, tag="t1")
                    nc.vector.tensor_mul(t1, sg[:, 0:NDC], tg)
                    t2 = pb.tile([128, NDC, BS], f32, tag="t2")
                    nc.gpsimd.tensor_mul(t2, sg[:, NDC:2 * NDC], c_state[s])
                    cn = pb.tile([128, NDC, BS], f32, tag="cn")
                    nc.vector.tensor_add(cn, t1, t2)
                    # c <- cn*m ; h <- o * tanh(c) (exact for binary masks:
                    # m=0 zeroes c so tanh(c)=0 zeroes h too)
                    m_sl = msk[:, :, :, tl]
                    nc.gpsimd.tensor_mul(c_state[s], cn, m_sl)
                    th = pb.tile([128, NDC, BS], f32, tag="th")
                    nc.scalar.activation(th, c_state[s], AF.Tanh)
                    nc.vector.tensor_mul(
                        hh_t[s][:, tl], sg[:, 2 * NDC:3 * NDC], th)
                    h_prev[s] = hh_t[s][:, tl]

            # -- cross-stream output gates (batched over the chunk) --
            if not _ge("full"):
                continue
            o_t = []
            for s in range(2):
                src = hh_t[1 - s]  # gate for stream s reads the OTHER h
                wgT = (wg0, wg1)[s]
                bgt = (bg0, bg1)[s]
                pg = ps.tile([128, NDC, BS, TC], f32, tag="lag")
                for go in range(NDC):
                    for dc in range(NDC):
                        nc.tensor.matmul(
                            pg[:, go],
                            lhsT=wgT[:, dc, go * 128:(go + 1) * 128],
                            rhs=src[:, :, dc, :].rearrange("p t b -> p b t"),
                            start=(dc == 0), stop=False)
                    nc.tensor.matmul(
                        pg[:, go], lhsT=bgt[0:1, go * 128:(go + 1) * 128],
                        rhs=ones16[0:1, 0:BS * TC],
                        start=False, stop=True)
                sp = pb.tile([128, NDC, BS, TC], f16, tag="sp")
                nc.scalar.activation(sp, pg, AF.Sigmoid)
                ot = pb.tile([128, NDC, BS, TC], f16, tag=f"o{s}")
                nc.vector.tensor_mul(
                    ot, sp, hh_t[s].rearrange("p t c b -> p c b t"))
                o_t.append(ot)

            # -- fusion: tanh(wf1.T @ o0 + wf2.T @ o1 + bf) --
            fp_ = ps.tile([128, NDC, BS, TC], f32, tag="lag")
            for do in range(NDC):
                first = True
                for s, wfT in enumerate((wf1, wf2)):
                    for dc in range(NDC):
                        nc.tensor.matmul(
                            fp_[:, do],
                            lhsT=wfT[:, dc, do * 128:(do + 1) * 128],
                            rhs=o_t[s][:, dc],
                            start=first, stop=False)
                        first = False
                nc.tensor.matmul(
                    fp_[:, do], lhsT=bf[0:1, do * 128:(do + 1) * 128],
                    rhs=ones16[0:1, 0:BS * TC],
                    start=False, stop=True)
            otn = pb.tile([128, NDC, BS, TC], f16, tag="otn")
            nc.scalar.activation(otn, fp_, AF.Tanh)

            # -- transpose back to natural layout and store --
            for bh in range(2):
                on = pb.tile([128, NDC, 128], f32, tag="on")
                for do in range(NDC):
                    tp2 = ps.tile([128, 128], f16, tag="g")
                    nc.tensor.transpose(
                        tp2, otn[:, do, bh * 8:(bh + 1) * 8, :], ident)
                    nc.vector.tensor_copy(on[:, do], tp2)
                nc.gpsimd.dma_start(
                    out=out_d.rearrange("b t (c p) -> b t c p", p=128)[
                        bh * 8:(bh + 1) * 8, t0:t0 + TC],
                    in_=on)

    nc.compile()
    return nc


def _prep_weights(i):
    """Host-side weight packing: fp16 casts, transposes, gate reorder."""
    def perm_gates_rows(w):  # [4D, ...] rows (i,f,g,o) -> (i,f,o,g)
        return np.concatenate(
            [w[0:D], w[D:2 * D], w[3 * D:4 * D], w[2 * D:3 * D]], axis=0)

    f16 = np.float16
    out = {}
    out["w_emb0T"] = np.ascontiguousarray(i["w_emb0"].T.astype(f16))
    out["w_emb1T"] = np.ascontiguousarray(i["w_emb1"].T.astype(f16))
    for s in range(2):
        out[f"w_ih{s}T"] = np.ascontiguousarray(
            perm_gates_rows(i[f"w_ih{s}"]).T.astype(f16))
        out[f"w_hh{s}T"] = np.ascontiguousarray(
            perm_gates_rows(i[f"w_hh{s}"]).T.astype(f16))
        bcs = perm_gates_rows(
            (i[f"b_ih{s}"] + i[f"b_hh{s}"]).reshape(4 * D, 1))[:, 0]
        out[f"bc{s}"] = bcs.astype(f16).reshape(1, G4)
        out[f"wg{s}T"] = np.ascontiguousarray(i[f"wg{s}"].T.astype(f16))
        out[f"bg{s}"] = i[f"bg{s}"].astype(f16).reshape(1, D)
    out["wf1T"] = np.ascontiguousarray(i["wf1"].T.astype(f16))
    out["wf2T"] = np.ascontiguousarray(i["wf2"].T.astype(f16))
    out["bf"] = i["bf"].astype(f16).reshape(1, D)
    for s in range(2):
        out[f"gamma{s}"] = i[f"gamma{s}"].astype(np.float32)
        out[f"beta{s}"] = i[f"beta{s}"].astype(np.float32)
    return out


def kernel(**inputs):
    from concourse.bass_utils import run_bass_kernel_spmd

    global _BUILT
    if _BUILT is None:
        _BUILT = _build(T)
    nc = _BUILT

    w = _prep_weights(inputs)
    in_maps = []
    for cid in range(NCORES):
        sl = slice(cid * BS, (cid + 1) * BS)
        m = dict(w)
        m["feat0"] = np.ascontiguousarray(
            inputs["feat0"][sl]).reshape(ROWS, F0)
        m["feat1"] = np.ascontiguousarray(
            inputs["feat1"][sl]).reshape(ROWS, F1)
        m["feat_mask"] = np.ascontiguousarray(
            inputs["feat_mask"][sl].astype(np.float32))
        in_maps.append(m)

    res = run_bass_kernel_spmd(nc, in_maps, core_ids=list(range(NCORES)))
    outs = [res.results[cid]["out"] for cid in range(NCORES)]
    return np.concatenate(outs, axis=0)


if __name__ == "__main__":
    nc = _build(T)
    print("built ok")



# revision 21
# speedup vs baseline: 1.1001x; 1.1001x over previous
"""Trainium2 Bass kernel for the two-stream LSTM encoder.

Strategy (8 NeuronCores, data-parallel over batch):
  - Each core gets B/8 = 16 batch elements; all weights replicated.
  - Everything on-device runs in "T-layout": channels on SBUF partitions,
    (batch, time) on the free dimension. fp16 matmul operands, fp32 psum
    accumulation and elementwise/state math (validated: rel err ~5e-4).
  - Phase A: z = feat @ w_emb.T streamed over row-chunks (PE transposes the
    activations into T-layout), per-channel sum/sumsq accumulated, raw z
    spilled to DRAM as fp16. BatchNorm statistics are all-reduced across the
    8 cores (training-mode BN over the full 32768-row batch).
  - Phase B: time loop in chunks of 16 steps. Per chunk: z chunk -> BN+ReLU
    (one ACT op per d-chunk, per-partition scale/bias) -> input projections
    u = e @ w_ih.T batched over the chunk (bias folded in via a K=1 matmul
    against a ones row). Per step: only the two h @ w_hh.T cell matmuls
    (64 weight-stationary 128x128 MMs each) + ACT sigmoid/tanh + DVE state
    update. Cross-stream output gates, the final fusion matmul and the
    transpose back to natural layout are batched per chunk (they lag the
    recurrence and hide in the PE array gaps).
Gate order is host-permuted from torch's (i,f,g,o) to (i,f,o,g) so one
sigmoid ACT covers a contiguous [128,192] region and one tanh covers [128,64].
"""

import os
import numpy as np

os.environ.setdefault("MYCRO_LOCAL_CACHE", "1")

NCORES = 8
B, T, D, F0, F1 = 128, 256, 512, 2048, 1024
BS = B // NCORES          # 16 batch rows per core
ROWS = BS * T             # 4096
TC = 16                   # time steps per chunk
NCHUNK = T // TC
EPS = 1e-5
G4 = 4 * D                # 2048 gate dim
NDC = D // 128            # 4 d-chunks
NGC = G4 // 128           # 16 gate chunks
NRC = ROWS // 128         # 32 row chunks
WS = 64.0                 # fp8 whh scale: gates computed as WS*(h@whh + ...)

_BUILT = None


def _build(t_steps=T, use_collective=True, stage="full"):
    import concourse.bass as bass
    import concourse.bacc as bacc
    import concourse.mybir as mybir
    import concourse.tile as tile
    from concourse.masks import make_identity
    from contextlib import ExitStack

    f16 = mybir.dt.float16
    f32 = mybir.dt.float32
    f8 = mybir.dt.float8e4
    AF = mybir.ActivationFunctionType
    ALU = mybir.AluOpType

    nchunk = t_steps // TC
    rows = BS * t_steps
    nrc = rows // 128

    nc = bacc.Bacc(None, num_devices=NCORES)

    # ---------------- DRAM parameters ----------------
    feat0 = nc.declare_dram_parameter("feat0", [rows, F0], f32, isOutput=False)
    feat1 = nc.declare_dram_parameter("feat1", [rows, F1], f32, isOutput=False)
    maskp = nc.declare_dram_parameter("feat_mask", [BS, t_steps], f32, isOutput=False)
    wemb0T = nc.declare_dram_parameter("w_emb0T", [F0, D], f16, isOutput=False)
    wemb1T = nc.declare_dram_parameter("w_emb1T", [F1, D], f16, isOutput=False)
    wih0T = nc.declare_dram_parameter("w_ih0T", [D, G4], f16, isOutput=False)
    whh0T = nc.declare_dram_parameter("w_hh0T", [D, G4], f8, isOutput=False)
    wih1T = nc.declare_dram_parameter("w_ih1T", [D, G4], f16, isOutput=False)
    whh1T = nc.declare_dram_parameter("w_hh1T", [D, G4], f8, isOutput=False)
    wg0T = nc.declare_dram_parameter("wg0T", [D, D], f16, isOutput=False)
    wg1T = nc.declare_dram_parameter("wg1T", [D, D], f16, isOutput=False)
    wf1T = nc.declare_dram_parameter("wf1T", [D, D], f16, isOutput=False)
    wf2T = nc.declare_dram_parameter("wf2T", [D, D], f16, isOutput=False)
    bc0p = nc.declare_dram_parameter("bc0", [1, G4], f16, isOutput=False)
    bc1p = nc.declare_dram_parameter("bc1", [1, G4], f16, isOutput=False)
    bg0p = nc.declare_dram_parameter("bg0", [1, D], f16, isOutput=False)
    bg1p = nc.declare_dram_parameter("bg1", [1, D], f16, isOutput=False)
    bfp = nc.declare_dram_parameter("bf", [1, D], f16, isOutput=False)
    gamma0p = nc.declare_dram_parameter("gamma0", [D], f32, isOutput=False)
    beta0p = nc.declare_dram_parameter("beta0", [D], f32, isOutput=False)
    gamma1p = nc.declare_dram_parameter("gamma1", [D], f32, isOutput=False)
    beta1p = nc.declare_dram_parameter("beta1", [D], f32, isOutput=False)
    out_d = nc.declare_dram_parameter("out", [BS, t_steps, D], f32, isOutput=True)

    # DRAM scratch: z in T-layout [dc, d_in_chunk, rows], fp16
    z0d = nc.dram_tensor("z0d", [NDC, 128, rows], f16)
    z1d = nc.dram_tensor("z1d", [NDC, 128, rows], f16)
    st_in = nc.dram_tensor("st_in", [128, 16], f32)
    st_out = nc.dram_tensor("st_out", [128, 16], f32)

    inv_n = 1.0 / float(B * t_steps)
    if not use_collective:
        inv_n = 1.0 / float(BS * t_steps)

    with tile.TileContext(nc) as tc, ExitStack() as stk:
        wp = stk.enter_context(tc.tile_pool(name="wp", bufs=1))

        # ---------------- resident small tiles ----------------
        ident = wp.tile([128, 128], f16)
        make_identity(nc, ident)
        ones16 = wp.tile([1, 512], f16)
        nc.vector.memset(ones16, 1.0)
        ones32 = wp.tile([1, 128], f32)
        nc.vector.memset(ones32, 1.0)
        eps_t = wp.tile([128, 1], f32)
        nc.vector.memset(eps_t, 0.0)

        # phase-B weight loads are deferred until after phase A's feat DMAs
        # are queued, so they don't steal HBM bandwidth from the critical
        # first feat tiles.
        def load_T(dram, nch, free, dt=f16):
            t = wp.tile([128, nch, free], dt, name=f"w_{dram.name}")
            nc.sync.dma_start(
                out=t, in_=dram.rearrange("(c p) g -> p c g", p=128))
            return t

        _STAGES = ("w", "feat", "tr", "mm", "a1", "a2", "a3", "a", "bn", "u", "steps", "full")
        def _ge(x):
            return _STAGES.index(stage) >= _STAGES.index(x)

        # per-rowchunk stat slots: col = dc*nrc + rc
        stat = {}
        for s in range(2):
            stat[(s, "s")] = wp.tile([128, NDC * nrc], f32, name=f"stat_s{s}")
            stat[(s, "q")] = wp.tile([128, NDC * nrc], f32, name=f"stat_q{s}")

        # ---------------- Phase A: embeddings + stats ----------------
        with tc.tile_pool(name="pa", bufs=2) as pa, \
             tc.tile_pool(name="paw", bufs=1) as paw, \
             tc.tile_pool(name="psA", bufs=1, space="PSUM") as psA:
            we0 = paw.tile([128, F0 // 128, D], f16)
            nc.sync.dma_start(
                out=we0, in_=wemb0T.rearrange("(c p) g -> p c g", p=128))
            we1 = paw.tile([128, F1 // 128, D], f16)
            nc.sync.dma_start(
                out=we1, in_=wemb1T.rearrange("(c p) g -> p c g", p=128))

            for s, (featp, nf, wemb, zdram) in enumerate(
                    ((feat0, F0, we0, z0d), (feat1, F1, we1, z1d))):
                nfc = nf // 128
                for rc in range(nrc if _ge("feat") else 0):
                    ftile = pa.tile([128, nf], f32, tag=f"ft{s}")
                    nc.gpsimd.dma_start(
                        out=ftile, in_=featp[rc * 128:(rc + 1) * 128, :])
                    f16t = pa.tile([128, nf], f16, tag=f"f16_{s}")
                    nc.scalar.copy(f16t, ftile)
                    if not _ge("tr"):
                        continue
                    fT = pa.tile([128, nfc, 128], f16, tag=f"fT{s}")
                    for fc in range(nfc):
                        tp = psA.tile([128, 128], f16, tag="tp", bufs=4)
                        nc.tensor.transpose(
                            tp, f16t[:, fc * 128:(fc + 1) * 128], ident)
                        nc.vector.tensor_copy(fT[:, fc], tp)
                    if not _ge("mm"):
                        continue
                    za = psA.tile([128, D], f32, tag="za", bufs=2)
                    for dc in range(NDC):
                        for fc in range(nfc):
                            nc.tensor.matmul(
                                za[:, dc * 128:(dc + 1) * 128],
                                lhsT=wemb[:, fc, dc * 128:(dc + 1) * 128],
                                rhs=fT[:, fc],
                                start=(fc == 0), stop=(fc == nfc - 1))
                    if not _ge("a1"):
                        continue
                    zst = pa.tile([128, NDC, 128], f16, tag=f"zst{s}")
                    sq = pa.tile([128, 128], f32, tag="sq")
                    for dc in range(NDC):
                        nc.scalar.activation(
                            zst[:, dc], za[:, dc * 128:(dc + 1) * 128],
                            AF.Identity,
                            accum_out=stat[(s, "s")][:, dc * nrc + rc:dc * nrc + rc + 1])
                        if _ge("a2"):
                            nc.vector.tensor_tensor(
                                sq, za[:, dc * 128:(dc + 1) * 128],
                                zst[:, dc], op=ALU.mult)
                            nc.vector.reduce_sum(
                                stat[(s, "q")][:, dc * nrc + rc:dc * nrc + rc + 1],
                                sq, axis=mybir.AxisListType.X)
                    if _ge("a3"):
                        nc.gpsimd.dma_start(
                            out=zdram[:, :, rc * 128:(rc + 1) * 128].rearrange(
                                "c p r -> p c r"),
                            in_=zst)

        # deferred phase-B weight DMAs (sync queue, overlap with phase A tail)
        wih0 = load_T(wih0T, NDC, G4)
        whh0 = load_T(whh0T, NDC, G4, f8)
        wih1 = load_T(wih1T, NDC, G4)
        whh1 = load_T(whh1T, NDC, G4, f8)
        wg0 = load_T(wg0T, NDC, D)
        wg1 = load_T(wg1T, NDC, D)
        wf1 = load_T(wf1T, NDC, D)
        wf2 = load_T(wf2T, NDC, D)
        # biases in per-partition layout [128, nchunks] for ACT bias folding
        bcs0 = wp.tile([128, NGC], f16)
        nc.sync.dma_start(out=bcs0, in_=bc0p.rearrange("a (c p) -> p (a c)", p=128))
        bcs1 = wp.tile([128, NGC], f16)
        nc.sync.dma_start(out=bcs1, in_=bc1p.rearrange("a (c p) -> p (a c)", p=128))
        bgs0 = wp.tile([128, NDC], f16)
        nc.sync.dma_start(out=bgs0, in_=bg0p.rearrange("a (c p) -> p (a c)", p=128))
        bgs1 = wp.tile([128, NDC], f16)
        nc.sync.dma_start(out=bgs1, in_=bg1p.rearrange("a (c p) -> p (a c)", p=128))
        bfs = wp.tile([128, NDC], f16)
        nc.sync.dma_start(out=bfs, in_=bfp.rearrange("a (c p) -> p (a c)", p=128))

        gam0 = wp.tile([128, NDC], f32)
        nc.sync.dma_start(out=gam0, in_=gamma0p.rearrange("(c p) -> p c", p=128))
        bet0 = wp.tile([128, NDC], f32)
        nc.sync.dma_start(out=bet0, in_=beta0p.rearrange("(c p) -> p c", p=128))
        gam1 = wp.tile([128, NDC], f32)
        nc.sync.dma_start(out=gam1, in_=gamma1p.rearrange("(c p) -> p c", p=128))
        bet1 = wp.tile([128, NDC], f32)
        nc.sync.dma_start(out=bet1, in_=beta1p.rearrange("(c p) -> p c", p=128))

        # mask (fp32, consumed by K=1 broadcast matmuls per chunk)
        mflat = wp.tile([1, BS, t_steps], f32)
        nc.sync.dma_start(out=mflat[0:1], in_=maskp[:, :])

        # ---------------- BN stats allreduce + scale/shift ----------------
        ared = wp.tile([128, 16], f32)
        for s in range(2 if _ge("a") else 0):
            nc.vector.reduce_sum(
                ared[:, s * 8:s * 8 + 4],
                stat[(s, "s")].rearrange("p (c r) -> p c r", c=NDC),
                axis=mybir.AxisListType.X)
            nc.vector.reduce_sum(
                ared[:, s * 8 + 4:s * 8 + 8],
                stat[(s, "q")].rearrange("p (c r) -> p c r", c=NDC),
                axis=mybir.AxisListType.X)
        if _ge("a"):
            nc.gpsimd.dma_start(out=st_in[:, :], in_=ared)
        if use_collective and _ge("bn"):
            nc.gpsimd.collective_compute(
                "AllReduce", ALU.add,
                replica_groups=[list(range(NCORES))],
                ins=[st_in[:, :]], outs=[st_out[:, :]])
        else:
            nc.gpsimd.dma_start(out=st_out[:, :], in_=st_in[:, :])
        ag = wp.tile([128, 16], f32)
        nc.gpsimd.dma_start(out=ag, in_=st_out[:, :])
        bn_on = _ge("bn")

        # a = gamma / sqrt(var+eps), c = beta - mu * a   (per stream)
        bn_a, bn_c = [], []
        for s, (gam, bet) in enumerate(
                ((gam0, bet0), (gam1, bet1)) if bn_on else ()):
            mu = wp.tile([128, NDC], f32, name=f"mu{s}")
            nc.vector.tensor_scalar_mul(mu, ag[:, s * 8:s * 8 + 4], inv_n)
            var = wp.tile([128, NDC], f32, name=f"var{s}")
            nc.vector.tensor_scalar_mul(var, ag[:, s * 8 + 4:s * 8 + 8], inv_n)
            musq = wp.tile([128, NDC], f32, name=f"musq{s}")
            nc.vector.tensor_mul(musq, mu, mu)
            nc.vector.tensor_sub(var, var, musq)
            nc.vector.tensor_scalar_add(var, var, EPS)
            sig = wp.tile([128, NDC], f32, name=f"sig{s}")
            nc.scalar.activation(sig, var, AF.Sqrt, bias=eps_t[:, 0:1])
            isig = wp.tile([128, NDC], f32, name=f"isig{s}")
            nc.vector.reciprocal(isig, sig)
            a_t = wp.tile([128, NDC], f32, name=f"bna{s}")
            nc.vector.tensor_mul(a_t, gam, isig)
            c_t = wp.tile([128, NDC], f32, name=f"bnc{s}")
            nc.vector.tensor_mul(c_t, mu, a_t)
            nc.vector.tensor_sub(c_t, bet, c_t)
            bn_a.append(a_t)
            bn_c.append(c_t)

        # ---------------- Phase B: recurrence ----------------
        pb = stk.enter_context(tc.tile_pool(name="pb", bufs=2))
        ps = stk.enter_context(tc.tile_pool(name="ps", bufs=2, space="PSUM"))

        # persistent state
        h_zero = wp.tile([128, NDC, BS], f16)
        nc.vector.memset(h_zero, 0.0)
        c_state = []
        for s in range(2):
            cs = wp.tile([128, NDC, BS], f32, name=f"cstate{s}")
            nc.vector.memset(cs, 0.0)
            c_state.append(cs)
        h_prev = [h_zero, h_zero]

        whh = (whh0, whh1)
        wih = (wih0, wih1)

        for c in range(nchunk if _ge("u") else 0):
            t0 = c * TC
            # -- load z chunks, BN+ReLU -> e (fp16) --
            ec = []
            for s, zdram in enumerate((z0d, z1d)):
                zc = pb.tile([128, NDC, BS, TC], f16, tag=f"zc{s}")
                for dc in range(NDC):
                    nc.sync.dma_start(
                        out=zc[:, dc],
                        in_=zdram.rearrange("c p (b t) -> p c b t", b=BS)[
                            :, dc, :, t0:t0 + TC])
                e = pb.tile([128, NDC, BS, TC], f16, tag=f"ec{s}")
                for dc in range(NDC):
                    nc.scalar.activation(
                        e[:, dc], zc[:, dc], AF.Relu,
                        bias=bn_c[s][:, dc:dc + 1],
                        scale=bn_a[s][:, dc:dc + 1])
                ec.append(e)

            # -- input projections u = e @ w_ih.T + bc (bias folded into the
            # PSUM->SBUF ACT copy as a per-partition bias) --
            u = []
            for s in range(2):
                ut = pb.tile([128, NGC, BS, TC], f16, tag=f"u{s}")
                bcs = (bcs0, bcs1)[s]
                for g in range(NGC):
                    up = ps.tile([128, BS, TC], f32, tag="u")
                    for dc in range(NDC):
                        nc.tensor.matmul(
                            up,
                            lhsT=wih[s][:, dc, g * 128:(g + 1) * 128],
                            rhs=ec[s][:, dc],
                            start=(dc == 0), stop=(dc == NDC - 1))
                    nc.scalar.activation(
                        ut[:, g], up, AF.Identity, bias=bcs[:, g:g + 1])
                u.append(ut)

            # -- mask broadcast for this chunk (fp32 K=1 matmuls, one per dc) --
            mp = ps.tile([128, NDC, BS, TC], f32, tag="lag")
            for dc in range(NDC):
                nc.tensor.matmul(
                    mp[:, dc],
                    lhsT=ones32[0:1, :],
                    rhs=mflat[0:1, :, t0:t0 + TC],
                    start=True, stop=True)
            msk = pb.tile([128, NDC, BS, TC], f16, tag="msk")
            nc.vector.tensor_copy(msk, mp)

            # -- recurrence steps --
            if not _ge("steps"):
                continue
            hh_t = [pb.tile([128, TC, NDC, BS], f16, tag="hh0", bufs=3,
                            name="hh0"),
                    pb.tile([128, TC, NDC, BS], f16, tag="hh1", bufs=3,
                            name="hh1")]
            for tl in range(TC):
                for s in range(2):
                    gp = ps.tile([128, NGC, BS], f32, tag="g")
                    for g in range(NGC):
                        for dc in range(NDC):
                            nc.tensor.matmul(
                                gp[:, g],
                                lhsT=whh[s][:, dc, g * 128:(g + 1) * 128],
                                rhs=h_prev[s][:, dc],
                                start=(dc == 0), stop=(dc == NDC - 1))
                    gsb = pb.tile([128, NGC, BS], f32, tag="gsb")
                    nc.vector.tensor_tensor(
                        gsb, gp, u[s][:, :, :, tl], op=ALU.add)
                    sg = pb.tile([128, 3 * NDC, BS], f32, tag="sg")
                    nc.scalar.activation(
                        sg, gsb[:, 0:3 * NDC], AF.Sigmoid, scale=1.0 / WS)
                    tg = pb.tile([128, NDC, BS], f32, tag="tg")
                    nc.scalar.activation(
                        tg, gsb[:, 3 * NDC:4 * NDC], AF.Tanh, scale=1.0 / WS)
                    t1 = pb.tile([128, NDC, BS], f32, tag="t1")
                    nc.vector.tensor_mul(t1, sg[:, 0:NDC], tg)
                    t2 = pb.tile([128, NDC, BS], f32, tag="t2")
                    nc.gpsimd.tensor_mul(t2, sg[:, NDC:2 * NDC], c_state[s])
                    cn = pb.tile([128, NDC, BS], f32, tag="cn")
                    nc.vector.tensor_add(cn, t1, t2)
                    # c <- cn*m ; h <- o * tanh(c) (exact for binary masks:
                    # m=0 zeroes c so tanh(c)=0 zeroes h too)
                    m_sl = msk[:, :, :, tl]
                    nc.gpsimd.tensor_mul(c_state[s], cn, m_sl)
                    th = pb.tile([128, NDC, BS], f32, tag="th")
                    nc.scalar.activation(th, c_state[s], AF.Tanh)
                    nc.vector.tensor_mul(
                        hh_t[s][:, tl], sg[:, 2 * NDC:3 * NDC], th)
                    h_prev[s] = hh_t[s][:, tl]

            # -- cross-stream output gates (batched over the chunk) --
            if not _ge("full"):
                continue
            o_t = []
            for s in range(2):
                src = hh_t[1 - s]  # gate for stream s reads the OTHER h
                wgT = (wg0, wg1)[s]
                bgs = (bgs0, bgs1)[s]
                pg = ps.tile([128, NDC, BS, TC], f32, tag="lag")
                sp = pb.tile([128, NDC, BS, TC], f16, tag="sp")
                for go in range(NDC):
                    for dc in range(NDC):
                        nc.tensor.matmul(
                            pg[:, go],
                            lhsT=wgT[:, dc, go * 128:(go + 1) * 128],
                            rhs=src[:, :, dc, :].rearrange("p t b -> p b t"),
                            start=(dc == 0), stop=(dc == NDC - 1))
                    nc.scalar.activation(
                        sp[:, go], pg[:, go], AF.Sigmoid,
                        bias=bgs[:, go:go + 1])
                ot = pb.tile([128, NDC, BS, TC], f16, tag=f"o{s}")
                nc.vector.tensor_mul(
                    ot, sp, hh_t[s].rearrange("p t c b -> p c b t"))
                o_t.append(ot)

            # -- fusion: tanh(wf1.T @ o0 + wf2.T @ o1 + bf) --
            fp_ = ps.tile([128, NDC, BS, TC], f32, tag="lag")
            otn = pb.tile([128, NDC, BS, TC], f16, tag="otn")
            for do in range(NDC):
                n_mm = 0
                for s, wfT in enumerate((wf1, wf2)):
                    for dc in range(NDC):
                        nc.tensor.matmul(
                            fp_[:, do],
                            lhsT=wfT[:, dc, do * 128:(do + 1) * 128],
                            rhs=o_t[s][:, dc],
                            start=(n_mm == 0), stop=(n_mm == 2 * NDC - 1))
                        n_mm += 1
                nc.scalar.activation(
                    otn[:, do], fp_[:, do], AF.Tanh, bias=bfs[:, do:do + 1])

            # -- transpose back to natural layout and store --
            for bh in range(2):
                on = pb.tile([128, NDC, 128], f32, tag="on")
                for do in range(NDC):
                    tp2 = ps.tile([128, 128], f16, tag="g")
                    nc.tensor.transpose(
                        tp2, otn[:, do, bh * 8:(bh + 1) * 8, :], ident)
                    nc.vector.tensor_copy(on[:, do], tp2)
                nc.gpsimd.dma_start(
                    out=out_d.rearrange("b t (c p) -> b t c p", p=128)[
                        bh * 8:(bh + 1) * 8, t0:t0 + TC],
                    in_=on)

    nc.compile()
    return nc


def _prep_weights(i):
    """Host-side weight packing: fp16 casts, transposes, gate reorder."""
    def perm_gates_rows(w):  # [4D, ...] rows (i,f,g,o) -> (i,f,o,g)
        return np.concatenate(
            [w[0:D], w[D:2 * D], w[3 * D:4 * D], w[2 * D:3 * D]], axis=0)

    import ml_dtypes

    f16 = np.float16
    out = {}
    out["w_emb0T"] = np.ascontiguousarray(i["w_emb0"].T.astype(f16))
    out["w_emb1T"] = np.ascontiguousarray(i["w_emb1"].T.astype(f16))
    for s in range(2):
        out[f"w_ih{s}T"] = np.ascontiguousarray(
            (perm_gates_rows(i[f"w_ih{s}"]).T * WS).astype(f16))
        out[f"w_hh{s}T"] = np.ascontiguousarray(
            np.clip(perm_gates_rows(i[f"w_hh{s}"]).T * WS, -224, 224)
            .astype(ml_dtypes.float8_e4m3fn))
        bcs = perm_gates_rows(
            (i[f"b_ih{s}"] + i[f"b_hh{s}"]).reshape(4 * D, 1))[:, 0]
        out[f"bc{s}"] = (bcs * WS).astype(f16).reshape(1, G4)
        out[f"wg{s}T"] = np.ascontiguousarray(i[f"wg{s}"].T.astype(f16))
        out[f"bg{s}"] = i[f"bg{s}"].astype(f16).reshape(1, D)
    out["wf1T"] = np.ascontiguousarray(i["wf1"].T.astype(f16))
    out["wf2T"] = np.ascontiguousarray(i["wf2"].T.astype(f16))
    out["bf"] = i["bf"].astype(f16).reshape(1, D)
    for s in range(2):
        out[f"gamma{s}"] = i[f"gamma{s}"].astype(np.float32)
        out[f"beta{s}"] = i[f"beta{s}"].astype(np.float32)
    return out


def kernel(**inputs):
    from concourse.bass_utils import run_bass_kernel_spmd

    global _BUILT
    if _BUILT is None:
        _BUILT = _build(T)
    nc = _BUILT

    w = _prep_weights(inputs)
    in_maps = []
    for cid in range(NCORES):
        sl = slice(cid * BS, (cid + 1) * BS)
        m = dict(w)
        m["feat0"] = np.ascontiguousarray(
            inputs["feat0"][sl]).reshape(ROWS, F0)
        m["feat1"] = np.ascontiguousarray(
            inputs["feat1"][sl]).reshape(ROWS, F1)
        m["feat_mask"] = np.ascontiguousarray(
            inputs["feat_mask"][sl].astype(np.float32))
        in_maps.append(m)

    res = run_bass_kernel_spmd(nc, in_maps, core_ids=list(range(NCORES)))
    outs = [res.results[cid]["out"] for cid in range(NCORES)]
    return np.concatenate(outs, axis=0)


if __name__ == "__main__":
    nc = _build(T)
    print("built ok")

